# revision 29
# baseline (speedup 1.0000x reference)
"""GPT-OSS MoE experts kernel for Trainium2 (8 NeuronCores, expert-parallel).

Strategy
--------
- Expert-parallel: core e owns expert e's weights (1/8 of total weight bytes,
  read exactly once -> memory-bound). Host does routing (gather tokens per
  expert), weight re-staging (slice expert, transpose to contraction-major
  [K, N] tile layout, cast fp16), and the final scatter-add combine. No
  collectives needed.
- The reference's per-32-block fp8 quant-dequant collapses exactly to
  "round each element to 4 significant bits (RTNE)": the block scale is a
  power of two (mantissa rounding is scale-invariant) and the +-448 clip can
  never bind by construction. On device this is 3 VectorE ops (Veltkamp
  split); the 4-significant-bit activation values are then EXACT in fp16.
- fp16 weights round at 2^-11; end-to-end error vs the f32 reference is
  ~7e-3 absmax-rel - dominated by quantization-boundary flips either way,
  and fp16 halves the weight traffic of this DMA-bound kernel.
- Form-B matmuls: weight [128, 128] tiles are the STATIONARY operand, ALL
  tokens ride the moving free dim (N = padded token count <= 512). Outputs
  land output-major ([n, tokens]), feeding layer 2 with no transposes.
- Biases ride free inside the GEMM: the activations carry a constant-1 row
  at contraction index 2880, the weights a bias row.
- Zero-padding traffic is trimmed: the contraction is 22 full 128-row
  k-tiles plus a resident 65-row tail (rows 2816-2879 + bias row), and the
  last 64-wide output n-tile is a separate resident "edge" tensor, so only
  real weight bytes cross HBM (~49.8 MB/core vs 52.0 padded).
- DMA discipline: all loads stream on the sync HWDGE ring in 1.4-2.9 MB
  coalesced slabs (measured ~420-450 GB/s aggregate).  Stream order matches
  PE consumption order (xt, nt0, nt1, tails/edge, quads) so PE starts at
  ~14 us with no fill bubble; the 64-wide edge tiles run while the first
  quad is on the wire.  y stores ride the same ring but are issued strictly
  after every weight load (no head-of-line blocking), and ysb has 5
  buffers - one per store batch - so a slow store completion can never
  back-pressure vector/PE.  Critical resource is DMA engine 0 (it also
  serves the ~210 KB instruction-stream paging); exec sits within a few us
  of its busy-time roofline.
"""

import functools
import sys

sys.path.insert(0, "/opt/trn_rl_repo")

import numpy as np

import concourse.bass as bass  # noqa: F401
import concourse.mybir as mybir
import concourse.tile as tile
from concourse import bacc
from concourse.bass_utils import run_bass_kernel_spmd

P = 128
H = 2880          # hidden dim
II = 2880         # intermediate dim (gate/up width)
NE = 8            # experts == cores
KT2 = 22          # full 128-row tiles over the contraction dim
TAIL = 65         # contraction tail rows: 2816..2879 real + bias row
NT = 23           # 128-tiles over the padded output dims (22 full + 64-wide)
VC = float(2 ** 20 + 1)   # Veltkamp constant: RTNE to 4 significant bits
MAXTOK = 512              # moving free-dim (= PSUM f32 bank) limit

f32 = mybir.dt.float32
f16 = mybir.dt.float16
AF = mybir.ActivationFunctionType
ALU = mybir.AluOpType


def _rtne4(x):
    """Round f32 elements to 4 significant bits, RTNE (== reference
    quant_dequant_fp8 up to e4m3-subnormal leftovers)."""
    c = np.float32(VC)
    t = (x * c).astype(np.float32)
    return (t - (t - x)).astype(np.float32)


@functools.lru_cache(maxsize=4)
def _build(ccap):
    """Per-core Bass program; ccap = padded token capacity (<= MAXTOK)."""
    nc = bacc.Bacc(None, target_bir_lowering=False)

    xt_d = nc.declare_dram_parameter("xt", [P, NT, ccap], f16, isOutput=False)
    w1s_d = nc.declare_dram_parameter("w1s", [2, P, 2, KT2, P], f16, isOutput=False)
    w1_d = nc.declare_dram_parameter("w1", [10, P, 2, 2, KT2, P], f16, isOutput=False)
    w2_d = nc.declare_dram_parameter("w2", [5, P, 4, KT2, P], f16, isOutput=False)
    w2s_d = nc.declare_dram_parameter("w2s", [2, P, KT2, P], f16, isOutput=False)
    wkt_d = nc.declare_dram_parameter("wkt", [TAIL, 3, H], f16, isOutput=False)
    wne_d = nc.declare_dram_parameter("wne", [P, 3, KT2, 64], f16, isOutput=False)
    wr_d = nc.declare_dram_parameter("wr", [P, ccap], f32, isOutput=False)
    y_d = nc.declare_dram_parameter("y", [P, NT, ccap], f16, isOutput=True)

    with tile.TileContext(nc) as tc:
        with (
            tc.tile_pool(name="consts", bufs=1) as consts,
            tc.tile_pool(name="wslab", bufs=6) as wpool,
            tc.tile_pool(name="tmp", bufs=2) as tmp,
            tc.tile_pool(name="psum", bufs=4, space="PSUM") as psum,
        ):
            # resident tensors; stream order matches PE consumption order:
            # xt, nt0 slab, nt1 slab, then the resident tail/edge tensors
            # (consumed by the nt22 block PE runs third), then the quads.
            xts = consts.tile([P, NT, ccap], f16, tag="xt", name="xt")
            nc.sync.dma_start(xts, xt_d[:])

            def load_slab(shape, src):
                s = wpool.tile(shape, f16, tag="wslab", name="wslab")
                nc.sync.dma_start(s, src)
                return s

            slab_nt0 = load_slab([P, 2, KT2, P], w1s_d[0])
            slab_nt1 = load_slab([P, 2, KT2, P], w1s_d[1])
            wkt = consts.tile([TAIL, 3, H], f16, tag="wkt", name="wkt")
            nc.sync.dma_start(wkt, wkt_d[:])
            wne = consts.tile([P, 3, KT2, 64], f16, tag="wne", name="wne")
            nc.sync.dma_start(wne, wne_d[:])
            wrep = consts.tile([P, ccap], f32, tag="wrep", name="wrep")
            nc.sync.dma_start(wrep, wr_d[:])
            interT = consts.tile([P, NT, ccap], f16, tag="interT", name="interT")

            # HAM warmup while the first slabs + xt stream in
            wtile = consts.tile([P, P], f16, tag="wtile", name="wtile")
            nc.vector.memset(wtile, 0.25)
            wup = psum.tile([P, ccap], f32, tag="ps_g", name="wup")
            for _ in range(36):
                nc.tensor.matmul(wup[:, :P], wtile, wtile,
                                 start=True, stop=True, skip_group_check=True)

            def swiglu_store(gps, ups, nt, w):
                """gate=min(G,7); up1=clip(U,-7,7)+1; x=gate*sig(1.702g)*up1;
                then rtne4 -> interT[:w, nt, :] (exact in fp16)."""
                gate = tmp.tile([P, ccap], f32, tag="t_gate", name="t_gate")
                nc.vector.tensor_scalar_min(gate[:w], gps[:w], 7.0)
                sig = tmp.tile([P, ccap], f32, tag="t_sig", name="t_sig")
                nc.scalar.activation(sig[:w], gate[:w], AF.Sigmoid, scale=1.702)
                up1 = tmp.tile([P, ccap], f32, tag="t_up", name="t_up")
                nc.vector.tensor_scalar(up1[:w], ups[:w], 1.0, -6.0, ALU.add, ALU.max)
                nc.vector.tensor_scalar_min(up1[:w], up1[:w], 8.0)
                nc.vector.tensor_mul(gate[:w], gate[:w], sig[:w])
                xv = tmp.tile([P, ccap], f32, tag="t_xv", name="t_xv")
                nc.vector.tensor_mul(xv[:w], gate[:w], up1[:w])
                tv = tmp.tile([P, ccap], f32, tag="t_tv", name="t_tv")
                nc.vector.tensor_scalar_mul(tv[:w], xv[:w], VC)
                nc.vector.tensor_sub(xv[:w], tv[:w], xv[:w])
                nc.vector.tensor_sub(interT[:w, nt, :], tv[:w], xv[:w])

            # ---- layer 1 + swiglu + rtne4, one 128-wide n-tile at a time ----
            xtail = xts[0:TAIL, KT2, :]

            def l1_block(slabg, slabu, nt):
                w = P if nt < 22 else 64
                gps = psum.tile([P, ccap], f32, tag="ps_g", name="ps_g")
                ups = psum.tile([P, ccap], f32, tag="ps_u", name="ps_u")
                for k in range(KT2):
                    nc.tensor.matmul(gps[:w], slabg[:, k, :], xts[:, k, :],
                                     start=(k == 0), stop=False)
                    nc.tensor.matmul(ups[:w], slabu[:, k, :], xts[:, k, :],
                                     start=(k == 0), stop=False)
                nc.tensor.matmul(gps[:w], wkt[:, 0, nt * P:nt * P + w], xtail,
                                 start=False, stop=True)
                nc.tensor.matmul(ups[:w], wkt[:, 1, nt * P:nt * P + w], xtail,
                                 start=False, stop=True)
                swiglu_store(gps, ups, nt, w)

            # nt0 and nt1 ride small lead slabs so PE starts ASAP; nt22 (64
            # real columns, resident edge/tail weights only) fills the gap
            # while the first 2.88 MB quad is still on the wire.
            l1_block(slab_nt0[:, 0], slab_nt0[:, 1], 0)
            l1_block(slab_nt1[:, 0], slab_nt1[:, 1], 1)
            l1_block(wne[:, 0], wne[:, 1], 22)
            # layer-2 bias row: i == II lives at tile 22, partition 64
            nc.vector.memset(interT[64:65, 22, :], 1.0)

            for q in range(10):
                slab = load_slab([P, 2, 2, KT2, P], w1_d[q])
                for t in (0, 1):
                    l1_block(slab[:, t, 0], slab[:, t, 1], 2 + 2 * q + t)

            # ---- layer 2 + routing-weight scale ----
            # All L2 slab loads are issued FIRST: y stores ride the same sync
            # HWDGE ring, and a compute-dependent store must never sit ahead
            # of a weight load in the ring's FIFO (head-of-line blocking).
            # hts 0..21: 5 quad slabs + 2 singles at the end (short tail).
            slabs2 = []
            for j in range(5):
                slabs2.append((load_slab([P, 4, KT2, P], w2_d[j]),
                               tuple(range(4 * j, 4 * j + 4))))
            for j in range(2):
                s2 = wpool.tile([P, 1, KT2, P], f16, tag="wslab", name="wslab")
                nc.sync.dma_start(s2[:, 0], w2s_d[j])
                slabs2.append((s2, (20 + j,)))

            # ht 22 (64 real rows) first: resident weights -> no DMA wait,
            # and its store leaves the critical path entirely.
            YB = 6
            itail = interT[0:TAIL, KT2, :]
            yps = psum.tile([P, ccap], f32, tag="ps_g", name="ps_g")
            for k in range(KT2):
                nc.tensor.matmul(yps[:64], wne[:, 2, k, :], interT[:, k, :],
                                 start=(k == 0), stop=False)
            nc.tensor.matmul(yps[:64], wkt[:, 2, 2816:2880], itail,
                             start=False, stop=True)
            ysb = tmp.tile([P, YB, ccap], f16, tag="ysb", name="ysb", bufs=6)
            nc.vector.tensor_mul(ysb[:64, 0, :], yps[:64], wrep[:64])
            nc.sync.dma_start(y_d[0:64, 22:23, :], ysb[0:64, 0:1, :])

            # store batches; the last two are 2-wide so the post-compute
            # tail holds only one tiny store
            bat = {}
            for lo_, hi_ in ((0, 5), (6, 11), (12, 17), (18, 19), (20, 21)):
                for h in range(lo_, hi_ + 1):
                    bat[h] = (lo_, hi_)
            for slab2, hts in slabs2:
                for s, ht in enumerate(hts):
                    yps = psum.tile([P, ccap], f32, tag="ps_g", name="ps_g")
                    for k in range(KT2):
                        nc.tensor.matmul(yps, slab2[:, s, k, :], interT[:, k, :],
                                         start=(k == 0), stop=False)
                    nc.tensor.matmul(yps, wkt[:, 2, ht * P:(ht + 1) * P], itail,
                                     start=False, stop=True)
                    lo, hi = bat[ht]
                    if ht == lo:
                        ysb = tmp.tile([P, YB, ccap], f16, tag="ysb",
                                       name="ysb", bufs=6)
                    nc.vector.tensor_mul(ysb[:, ht - lo, :], yps, wrep)
                    if ht == hi:
                        nc.sync.dma_start(y_d[:, lo:ht + 1, :],
                                          ysb[:, :ht + 1 - lo, :])

    nc.finalize()
    return nc


def _stage(inputs):
    """Host-side routing + weight re-staging. Returns (nc, in_maps, assigns, T)."""
    hs = np.ascontiguousarray(np.asarray(inputs["hidden_states"], dtype=np.float32))
    ri = np.asarray(inputs["router_indices"]).astype(np.int64)
    rw = np.asarray(inputs["routing_weights"], dtype=np.float32)
    gup = np.asarray(inputs["gate_up_proj"], dtype=np.float32)
    gub = np.asarray(inputs["gate_up_proj_bias"], dtype=np.float32)
    dn = np.asarray(inputs["down_proj"], dtype=np.float32)
    dnb = np.asarray(inputs["down_proj_bias"], dtype=np.float32)

    T = hs.shape[0]
    topk = ri.shape[1]

    flat_e = ri.reshape(-1)
    order = np.argsort(flat_e, kind="stable")
    counts = np.bincount(flat_e, minlength=NE)
    starts = np.zeros(NE + 1, np.int64)
    starts[1:] = np.cumsum(counts)
    maxc = int(counts.max())
    # Each pass handles up to MAXTOK tokens per expert (seed-0 loads are ~142,
    # so this is a single pass; multiple passes only for pathological routing).
    npass = max(1, -(-maxc // MAXTOK))
    percap = -(-maxc // npass)
    # 32-granularity keeps the moving free dim >= ~160 for seed-0 loads;
    # shrinking it further makes matmuls LDWEIGHTS-bound (slower, measured).
    ccap = max(32, -(-percap // 32) * 32)

    x_dq = _rtne4(hs).astype(np.float16)   # 4-sig-bit values: exact in fp16
    rw_flat = rw.reshape(-1)

    def body(mat_t):
        # [2816, 2816] -> [nt, p(k), kt, q(n)] so each slab is one contiguous run
        return mat_t[:2816, :2816].reshape(22, P, 22, P).transpose(2, 1, 0, 3)

    def edge(mat_t):
        # k body rows x last 64 output cols -> [p(k), kt, q]
        return mat_t[:2816, 2816:2880].reshape(22, P, 64).transpose(1, 0, 2)

    def ktail(mat_t, bias):
        # contraction rows 2816..2879 + bias row -> [65, 2880]
        return np.concatenate([mat_t[2816:2880, :2880], bias[None, :2880]], axis=0)

    weights = []
    for e in range(NE):
        g = np.ascontiguousarray(gup[e, 0::2, :].T).astype(np.float16)
        u = np.ascontiguousarray(gup[e, 1::2, :].T).astype(np.float16)
        d = np.ascontiguousarray(dn[e].T).astype(np.float16)
        gb, ub, db = (gub[e, 0::2].astype(np.float16), gub[e, 1::2].astype(np.float16),
                      dnb[e].astype(np.float16))
        # w1s: [2, P, 2(g/u), 22, P] lead singles (nt 0, 1; 1.44 MB each)
        # w1: [10, P, 2(nt), 2(g/u), 22, P] quad slabs (nts 2..21; 2.88 MB)
        st = np.stack([body(g), body(u)], axis=2)
        w1s = np.ascontiguousarray(st[:2])
        w1 = np.ascontiguousarray(
            st[2:].reshape(10, 2, P, 2, 22, P).transpose(0, 2, 1, 3, 4, 5))
        bd = body(d)
        # w2: [5, P, 4(ht), 22, P] quads for hts 0..19 + 2 singles (20, 21)
        w2 = np.ascontiguousarray(
            bd[:20].reshape(5, 4, P, 22, P).transpose(0, 2, 1, 3, 4))
        w2s = np.ascontiguousarray(bd[20:22])
        wkt = np.ascontiguousarray(
            np.stack([ktail(g, gb), ktail(u, ub), ktail(d, db)], axis=1))
        wne = np.ascontiguousarray(np.stack([edge(g), edge(u), edge(d)], axis=1))
        weights.append((w1s, w1, w2, w2s, wkt, wne))

    passes, assigns = [], []
    for p in range(npass):
        in_maps, passigns = [], []
        for e in range(NE):
            a_all = order[starts[e] : starts[e + 1]]
            a = a_all[p * ccap : (p + 1) * ccap]
            toks = a // topk
            ce = len(a)
            passigns.append((a, toks))

            xt = np.zeros((NT * P, ccap), np.float16)
            xt[:H, :ce] = x_dq[toks].T
            xt[H, :] = np.float16(1.0)
            xt = np.ascontiguousarray(xt.reshape(NT, P, ccap).transpose(1, 0, 2))

            wr_rep = np.zeros((P, ccap), np.float32)
            wr_rep[:, :ce] = rw_flat[a][None, :]

            w1s, w1, w2, w2s, wkt, wne = weights[e]
            in_maps.append(dict(xt=xt, w1s=w1s, w1=w1, w2=w2, w2s=w2s,
                                wkt=wkt, wne=wne, wr=wr_rep))
        passes.append(in_maps)
        assigns.append(passigns)

    nc = _build(ccap)
    return nc, passes, assigns, T


def kernel(**inputs):
    nc, passes, assigns, T = _stage(inputs)
    out = np.zeros((T, H), np.float32)
    for in_maps, passigns in zip(passes, assigns):
        res = run_bass_kernel_spmd(nc, in_maps, list(range(NE)))
        for e in range(NE):
            a, toks = passigns[e]
            if len(a):
                yv = res.results[e]["y"]  # [P, NT, ccap]
                yt = yv.transpose(1, 0, 2).reshape(NT * P, -1)[:H, : len(a)]
                np.add.at(out, toks, yt.T.astype(np.float32))
    return out


# revision 30
# speedup vs baseline: 1.0010x; 1.0010x over previous
"""GPT-OSS MoE experts kernel for Trainium2 (8 NeuronCores, expert-parallel).

Strategy
--------
- Expert-parallel: core e owns expert e's weights (1/8 of total weight bytes,
  read exactly once -> memory-bound). Host does routing (gather tokens per
  expert), weight re-staging (slice expert, transpose to contraction-major
  [K, N] tile layout, cast fp16), and the final scatter-add combine. No
  collectives needed.
- The reference's per-32-block fp8 quant-dequant collapses exactly to
  "round each element to 4 significant bits (RTNE)": the block scale is a
  power of two (mantissa rounding is scale-invariant) and the +-448 clip can
  never bind by construction. On device this is 3 VectorE ops (Veltkamp
  split); the 4-significant-bit activation values are then EXACT in fp16.
- fp16 weights round at 2^-11; end-to-end error vs the f32 reference is
  ~7e-3 absmax-rel - dominated by quantization-boundary flips either way,
  and fp16 halves the weight traffic of this DMA-bound kernel.
- Form-B matmuls: weight [128, 128] tiles are the STATIONARY operand, ALL
  tokens ride the moving free dim (N = padded token count <= 512). Outputs
  land output-major ([n, tokens]), feeding layer 2 with no transposes.
- Biases ride free inside the GEMM: the activations carry a constant-1 row
  at contraction index 2880, the weights a bias row.
- Zero-padding traffic is trimmed: the contraction is 22 full 128-row
  k-tiles plus a resident 65-row tail (rows 2816-2879 + bias row), and the
  last 64-wide output n-tile is a separate resident "edge" tensor, so only
  real weight bytes cross HBM (~49.8 MB/core vs 52.0 padded).
- DMA discipline: all loads stream on the sync HWDGE ring in 1.4-2.9 MB
  coalesced slabs (measured ~420-450 GB/s aggregate).  Stream order matches
  PE consumption order (xt, nt0, nt1, tails/edge, quads) so PE starts at
  ~14 us with no fill bubble; the 64-wide edge tiles run while the first
  quad is on the wire.  y stores ride the same ring but are issued strictly
  after every weight load (no head-of-line blocking), and ysb has 5
  buffers - one per store batch - so a slow store completion can never
  back-pressure vector/PE.  Critical resource is DMA engine 0 (it also
  serves the ~210 KB instruction-stream paging); exec sits within a few us
  of its busy-time roofline.
"""

import functools
import sys

sys.path.insert(0, "/opt/trn_rl_repo")

import numpy as np

import concourse.bass as bass  # noqa: F401
import concourse.mybir as mybir
import concourse.tile as tile
from concourse import bacc
from concourse.bass_utils import run_bass_kernel_spmd

P = 128
H = 2880          # hidden dim
II = 2880         # intermediate dim (gate/up width)
NE = 8            # experts == cores
KT2 = 22          # full 128-row tiles over the contraction dim
TAIL = 65         # contraction tail rows: 2816..2879 real + bias row
NT = 23           # 128-tiles over the padded output dims (22 full + 64-wide)
VC = float(2 ** 20 + 1)   # Veltkamp constant: RTNE to 4 significant bits
MAXTOK = 512              # moving free-dim (= PSUM f32 bank) limit

f32 = mybir.dt.float32
f16 = mybir.dt.float16
AF = mybir.ActivationFunctionType
ALU = mybir.AluOpType


def _rtne4(x):
    """Round f32 elements to 4 significant bits, RTNE (== reference
    quant_dequant_fp8 up to e4m3-subnormal leftovers)."""
    c = np.float32(VC)
    t = (x * c).astype(np.float32)
    return (t - (t - x)).astype(np.float32)


@functools.lru_cache(maxsize=4)
def _build(ccap):
    """Per-core Bass program; ccap = padded token capacity (<= MAXTOK)."""
    nc = bacc.Bacc(None, target_bir_lowering=False)

    xt_d = nc.declare_dram_parameter("xt", [P, NT, ccap], f16, isOutput=False)
    w1s_d = nc.declare_dram_parameter("w1s", [2, P, 2, KT2, P], f16, isOutput=False)
    w1_d = nc.declare_dram_parameter("w1", [10, P, 2, 2, KT2, P], f16, isOutput=False)
    w2_d = nc.declare_dram_parameter("w2", [5, P, 4, KT2, P], f16, isOutput=False)
    w2s_d = nc.declare_dram_parameter("w2s", [2, P, KT2, P], f16, isOutput=False)
    wkt_d = nc.declare_dram_parameter("wkt", [TAIL, 3, H], f16, isOutput=False)
    wne_d = nc.declare_dram_parameter("wne", [P, 3, KT2, 64], f16, isOutput=False)
    wr_d = nc.declare_dram_parameter("wr", [P, ccap], f32, isOutput=False)
    y_d = nc.declare_dram_parameter("y", [P, NT, ccap], f16, isOutput=True)

    with tile.TileContext(nc) as tc:
        with (
            tc.tile_pool(name="consts", bufs=1) as consts,
            tc.tile_pool(name="wslab", bufs=6) as wpool,
            tc.tile_pool(name="tmp", bufs=2) as tmp,
            tc.tile_pool(name="psum", bufs=4, space="PSUM") as psum,
        ):
            # resident tensors; stream order matches PE consumption order:
            # xt, nt0 slab, nt1 slab, then the resident tail/edge tensors
            # (consumed by the nt22 block PE runs third), then the quads.
            xts = consts.tile([P, NT, ccap], f16, tag="xt", name="xt")
            nc.sync.dma_start(xts, xt_d[:])

            def load_slab(shape, src):
                s = wpool.tile(shape, f16, tag="wslab", name="wslab")
                nc.sync.dma_start(s, src)
                return s

            slab_nt0 = load_slab([P, 2, KT2, P], w1s_d[0])
            slab_nt1 = load_slab([P, 2, KT2, P], w1s_d[1])
            wkt = consts.tile([TAIL, 3, H], f16, tag="wkt", name="wkt")
            nc.sync.dma_start(wkt, wkt_d[:])
            wne = consts.tile([P, 3, KT2, 64], f16, tag="wne", name="wne")
            nc.sync.dma_start(wne, wne_d[:])
            wrep = consts.tile([P, ccap], f32, tag="wrep", name="wrep")
            nc.sync.dma_start(wrep, wr_d[:])
            interT = consts.tile([P, NT, ccap], f16, tag="interT", name="interT")

            # HAM warmup while the first slabs + xt stream in
            wtile = consts.tile([P, P], f16, tag="wtile", name="wtile")
            nc.vector.memset(wtile, 0.25)
            # 72 warmup matmuls bridge PE from program start (~7 us) to the
            # first slab arrival (~13.5 us): 36 cold-clock + 36 warm keep the
            # HAM window busy so the first REAL matmuls run at 2.4 GHz
            # (36 was measured to leave a >3.4 us idle gap -> PE re-colds).
            wup = psum.tile([P, ccap], f32, tag="ps_g", name="wup")
            for _ in range(72):
                nc.tensor.matmul(wup[:, :P], wtile, wtile,
                                 start=True, stop=True, skip_group_check=True)

            def swiglu_store(gps, ups, nt, w):
                """gate=min(G,7); up1=clip(U,-7,7)+1; x=gate*sig(1.702g)*up1;
                then rtne4 -> interT[:w, nt, :] (exact in fp16)."""
                gate = tmp.tile([P, ccap], f32, tag="t_gate", name="t_gate")
                nc.vector.tensor_scalar_min(gate[:w], gps[:w], 7.0)
                sig = tmp.tile([P, ccap], f32, tag="t_sig", name="t_sig")
                nc.scalar.activation(sig[:w], gate[:w], AF.Sigmoid, scale=1.702)
                up1 = tmp.tile([P, ccap], f32, tag="t_up", name="t_up")
                nc.vector.tensor_scalar(up1[:w], ups[:w], 1.0, -6.0, ALU.add, ALU.max)
                nc.vector.tensor_scalar_min(up1[:w], up1[:w], 8.0)
                nc.vector.tensor_mul(gate[:w], gate[:w], sig[:w])
                xv = tmp.tile([P, ccap], f32, tag="t_xv", name="t_xv")
                nc.vector.tensor_mul(xv[:w], gate[:w], up1[:w])
                tv = tmp.tile([P, ccap], f32, tag="t_tv", name="t_tv")
                nc.vector.tensor_scalar_mul(tv[:w], xv[:w], VC)
                nc.vector.tensor_sub(xv[:w], tv[:w], xv[:w])
                nc.vector.tensor_sub(interT[:w, nt, :], tv[:w], xv[:w])

            # ---- layer 1 + swiglu + rtne4, one 128-wide n-tile at a time ----
            xtail = xts[0:TAIL, KT2, :]

            def l1_block(slabg, slabu, nt):
                w = P if nt < 22 else 64
                gps = psum.tile([P, ccap], f32, tag="ps_g", name="ps_g")
                ups = psum.tile([P, ccap], f32, tag="ps_u", name="ps_u")
                for k in range(KT2):
                    nc.tensor.matmul(gps[:w], slabg[:, k, :], xts[:, k, :],
                                     start=(k == 0), stop=False)
                    nc.tensor.matmul(ups[:w], slabu[:, k, :], xts[:, k, :],
                                     start=(k == 0), stop=False)
                nc.tensor.matmul(gps[:w], wkt[:, 0, nt * P:nt * P + w], xtail,
                                 start=False, stop=True)
                nc.tensor.matmul(ups[:w], wkt[:, 1, nt * P:nt * P + w], xtail,
                                 start=False, stop=True)
                swiglu_store(gps, ups, nt, w)

            # nt0 and nt1 ride small lead slabs so PE starts ASAP; nt22 (64
            # real columns, resident edge/tail weights only) fills the gap
            # while the first 2.88 MB quad is still on the wire.
            l1_block(slab_nt0[:, 0], slab_nt0[:, 1], 0)
            l1_block(slab_nt1[:, 0], slab_nt1[:, 1], 1)
            l1_block(wne[:, 0], wne[:, 1], 22)
            # layer-2 bias row: i == II lives at tile 22, partition 64
            nc.vector.memset(interT[64:65, 22, :], 1.0)

            for q in range(10):
                slab = load_slab([P, 2, 2, KT2, P], w1_d[q])
                for t in (0, 1):
                    l1_block(slab[:, t, 0], slab[:, t, 1], 2 + 2 * q + t)

            # ---- layer 2 + routing-weight scale ----
            # All L2 slab loads are issued FIRST: y stores ride the same sync
            # HWDGE ring, and a compute-dependent store must never sit ahead
            # of a weight load in the ring's FIFO (head-of-line blocking).
            # hts 0..21: 5 quad slabs + 2 singles at the end (short tail).
            slabs2 = []
            for j in range(5):
                slabs2.append((load_slab([P, 4, KT2, P], w2_d[j]),
                               tuple(range(4 * j, 4 * j + 4))))
            for j in range(2):
                s2 = wpool.tile([P, 1, KT2, P], f16, tag="wslab", name="wslab")
                nc.sync.dma_start(s2[:, 0], w2s_d[j])
                slabs2.append((s2, (20 + j,)))

            # ht 22 (64 real rows) first: resident weights -> no DMA wait,
            # and its store leaves the critical path entirely.
            YB = 6
            itail = interT[0:TAIL, KT2, :]
            yps = psum.tile([P, ccap], f32, tag="ps_g", name="ps_g")
            for k in range(KT2):
                nc.tensor.matmul(yps[:64], wne[:, 2, k, :], interT[:, k, :],
                                 start=(k == 0), stop=False)
            nc.tensor.matmul(yps[:64], wkt[:, 2, 2816:2880], itail,
                             start=False, stop=True)
            ysb = tmp.tile([P, YB, ccap], f16, tag="ysb", name="ysb", bufs=6)
            nc.vector.tensor_mul(ysb[:64, 0, :], yps[:64], wrep[:64])
            nc.sync.dma_start(y_d[0:64, 22:23, :], ysb[0:64, 0:1, :])

            # store batches; the last two are 2-wide so the post-compute
            # tail holds only one tiny store
            bat = {}
            for lo_, hi_ in ((0, 5), (6, 11), (12, 17), (18, 19), (20, 21)):
                for h in range(lo_, hi_ + 1):
                    bat[h] = (lo_, hi_)
            for slab2, hts in slabs2:
                for s, ht in enumerate(hts):
                    yps = psum.tile([P, ccap], f32, tag="ps_g", name="ps_g")
                    for k in range(KT2):
                        nc.tensor.matmul(yps, slab2[:, s, k, :], interT[:, k, :],
                                         start=(k == 0), stop=False)
                    nc.tensor.matmul(yps, wkt[:, 2, ht * P:(ht + 1) * P], itail,
                                     start=False, stop=True)
                    lo, hi = bat[ht]
                    if ht == lo:
                        ysb = tmp.tile([P, YB, ccap], f16, tag="ysb",
                                       name="ysb", bufs=6)
                    nc.vector.tensor_mul(ysb[:, ht - lo, :], yps, wrep)
                    if ht == hi:
                        nc.sync.dma_start(y_d[:, lo:ht + 1, :],
                                          ysb[:, :ht + 1 - lo, :])

    nc.finalize()
    return nc


def _stage(inputs):
    """Host-side routing + weight re-staging. Returns (nc, in_maps, assigns, T)."""
    hs = np.ascontiguousarray(np.asarray(inputs["hidden_states"], dtype=np.float32))
    ri = np.asarray(inputs["router_indices"]).astype(np.int64)
    rw = np.asarray(inputs["routing_weights"], dtype=np.float32)
    gup = np.asarray(inputs["gate_up_proj"], dtype=np.float32)
    gub = np.asarray(inputs["gate_up_proj_bias"], dtype=np.float32)
    dn = np.asarray(inputs["down_proj"], dtype=np.float32)
    dnb = np.asarray(inputs["down_proj_bias"], dtype=np.float32)

    T = hs.shape[0]
    topk = ri.shape[1]

    flat_e = ri.reshape(-1)
    order = np.argsort(flat_e, kind="stable")
    counts = np.bincount(flat_e, minlength=NE)
    starts = np.zeros(NE + 1, np.int64)
    starts[1:] = np.cumsum(counts)
    maxc = int(counts.max())
    # Each pass handles up to MAXTOK tokens per expert (seed-0 loads are ~142,
    # so this is a single pass; multiple passes only for pathological routing).
    npass = max(1, -(-maxc // MAXTOK))
    percap = -(-maxc // npass)
    # 32-granularity keeps the moving free dim >= ~160 for seed-0 loads;
    # shrinking it further makes matmuls LDWEIGHTS-bound (slower, measured).
    ccap = max(32, -(-percap // 32) * 32)

    x_dq = _rtne4(hs).astype(np.float16)   # 4-sig-bit values: exact in fp16
    rw_flat = rw.reshape(-1)

    def body(mat_t):
        # [2816, 2816] -> [nt, p(k), kt, q(n)] so each slab is one contiguous run
        return mat_t[:2816, :2816].reshape(22, P, 22, P).transpose(2, 1, 0, 3)

    def edge(mat_t):
        # k body rows x last 64 output cols -> [p(k), kt, q]
        return mat_t[:2816, 2816:2880].reshape(22, P, 64).transpose(1, 0, 2)

    def ktail(mat_t, bias):
        # contraction rows 2816..2879 + bias row -> [65, 2880]
        return np.concatenate([mat_t[2816:2880, :2880], bias[None, :2880]], axis=0)

    weights = []
    for e in range(NE):
        g = np.ascontiguousarray(gup[e, 0::2, :].T).astype(np.float16)
        u = np.ascontiguousarray(gup[e, 1::2, :].T).astype(np.float16)
        d = np.ascontiguousarray(dn[e].T).astype(np.float16)
        gb, ub, db = (gub[e, 0::2].astype(np.float16), gub[e, 1::2].astype(np.float16),
                      dnb[e].astype(np.float16))
        # w1s: [2, P, 2(g/u), 22, P] lead singles (nt 0, 1; 1.44 MB each)
        # w1: [10, P, 2(nt), 2(g/u), 22, P] quad slabs (nts 2..21; 2.88 MB)
        st = np.stack([body(g), body(u)], axis=2)
        w1s = np.ascontiguousarray(st[:2])
        w1 = np.ascontiguousarray(
            st[2:].reshape(10, 2, P, 2, 22, P).transpose(0, 2, 1, 3, 4, 5))
        bd = body(d)
        # w2: [5, P, 4(ht), 22, P] quads for hts 0..19 + 2 singles (20, 21)
        w2 = np.ascontiguousarray(
            bd[:20].reshape(5, 4, P, 22, P).transpose(0, 2, 1, 3, 4))
        w2s = np.ascontiguousarray(bd[20:22])
        wkt = np.ascontiguousarray(
            np.stack([ktail(g, gb), ktail(u, ub), ktail(d, db)], axis=1))
        wne = np.ascontiguousarray(np.stack([edge(g), edge(u), edge(d)], axis=1))
        weights.append((w1s, w1, w2, w2s, wkt, wne))

    passes, assigns = [], []
    for p in range(npass):
        in_maps, passigns = [], []
        for e in range(NE):
            a_all = order[starts[e] : starts[e + 1]]
            a = a_all[p * ccap : (p + 1) * ccap]
            toks = a // topk
            ce = len(a)
            passigns.append((a, toks))

            xt = np.zeros((NT * P, ccap), np.float16)
            xt[:H, :ce] = x_dq[toks].T
            xt[H, :] = np.float16(1.0)
            xt = np.ascontiguousarray(xt.reshape(NT, P, ccap).transpose(1, 0, 2))

            wr_rep = np.zeros((P, ccap), np.float32)
            wr_rep[:, :ce] = rw_flat[a][None, :]

            w1s, w1, w2, w2s, wkt, wne = weights[e]
            in_maps.append(dict(xt=xt, w1s=w1s, w1=w1, w2=w2, w2s=w2s,
                                wkt=wkt, wne=wne, wr=wr_rep))
        passes.append(in_maps)
        assigns.append(passigns)

    nc = _build(ccap)
    return nc, passes, assigns, T


def kernel(**inputs):
    nc, passes, assigns, T = _stage(inputs)
    out = np.zeros((T, H), np.float32)
    for in_maps, passigns in zip(passes, assigns):
        res = run_bass_kernel_spmd(nc, in_maps, list(range(NE)))
        for e in range(NE):
            a, toks = passigns[e]
            if len(a):
                yv = res.results[e]["y"]  # [P, NT, ccap]
                yt = yv.transpose(1, 0, 2).reshape(NT * P, -1)[:H, : len(a)]
                np.add.at(out, toks, yt.T.astype(np.float32))
    return out
